# revision 8
# baseline (speedup 1.0000x reference)
"""Trainium2 Bass kernel for nn_CEBlock (topk_masking).

Data-parallel over batch: 32 samples -> 8 cores x 4 samples.
Per core, per sample:
  LN1 -> qkv (k & template-q in fp32, bulk q/v fp16) -> self-attention
  (attn output fp32-precise for template rows, fp16 elsewhere) ->
  proj residual -> CE top-k ranking (pairwise-compare ranks, exact
  stable ties) -> permutation-matmul gather -> cross-attention over
  ps (fp16) -> exact-GELU MLP (fp16) -> outputs.
"""
import math
import numpy as np

import concourse.bacc as bacc
import concourse.mybir as mybir
from concourse.tile import TileContext
from concourse import bass_utils

AF = mybir.ActivationFunctionType
ALU = mybir.AluOpType
AX = mybir.AxisListType
f32 = mybir.dt.float32
f16 = mybir.dt.float16
i32 = mybir.dt.int32

B, NT, NS, NPS, D, H = 32, 64, 256, 256, 768, 12
DH = D // H
LENS_KEEP = math.ceil(0.7 * NS)  # 180
N = NT + NS                      # 320
NK = NT + LENS_KEEP              # 244
EPS = 1e-5
SCALE = DH ** -0.5               # 0.125
NSAMP = 4
ND = D // 128                    # 6

TT = [(0, 64), (64, 192), (192, 320)]        # residual tiles
KT = [(0, 128), (128, 256), (256, 320)]      # key/q-row tiles
ST = [(64, 192), (192, 244)]                 # kept-token rows in y
SL = [(0, 128), (128, 180)]                  # kept-token local tiling
YT = [(0, 128), (128, 244)]                  # MLP row tiles


def _ln_normalize(nc, pool, tiles, sizes, tag, s, eps_t):
    out = []
    for i, (t, sz) in enumerate(zip(tiles, sizes)):
        st6 = pool.tile([128, 12], f32, tag=f"{tag}_st6", name=f"{tag}_st6_{i}_{s}")
        st2 = pool.tile([128, 2], f32, tag=f"{tag}_st2", name=f"{tag}_st2_{i}_{s}")
        nc.vector.bn_stats(st6[:sz, 0:6], t[:sz, 0:384])
        nc.vector.bn_stats(st6[:sz, 6:12], t[:sz, 384:768])
        nc.vector.bn_aggr(st2[:sz], st6[:sz])
        sd = pool.tile([128, 1], f32, tag=f"{tag}_sd", name=f"{tag}_sd_{i}_{s}")
        nc.scalar.activation(sd[:sz], st2[:sz, 1:2], AF.Sqrt, bias=eps_t[:sz])
        rs = pool.tile([128, 1], f32, tag=f"{tag}_rs", name=f"{tag}_rs_{i}_{s}")
        nc.vector.reciprocal(rs[:sz], sd[:sz])
        nmrs = pool.tile([128, 1], f32, tag=f"{tag}_nm", name=f"{tag}_nm_{i}_{s}")
        nc.vector.tensor_tensor(out=nmrs[:sz], in0=st2[:sz, 0:1], in1=rs[:sz], op=ALU.mult)
        nc.vector.tensor_scalar(nmrs[:sz], nmrs[:sz], -1.0, scalar2=None, op0=ALU.mult)
        h = pool.tile([128, 768], f32, tag=f"{tag}_h{i}", name=f"{tag}_h_{i}_{s}")
        nc.scalar.activation(h[:sz], t[:sz], AF.Identity, bias=nmrs[:sz], scale=rs[:sz])
        out.append(h)
    return out


def _transpose_cols(nc, pool, psp, ident, src_tiles, ranges, n_tok, tag, s,
                    out16=True, out32=False):
    t16 = [pool.tile([128, n_tok], f16, tag=f"{tag}16_{d}", name=f"{tag}16_{d}_{s}")
           for d in range(ND)] if out16 else None
    t32 = [pool.tile([128, n_tok], f32, tag=f"{tag}32_{d}", name=f"{tag}32_{d}_{s}")
           for d in range(ND)] if out32 else None
    for (a, b), src in zip(ranges, src_tiles):
        sz = b - a
        for d in range(ND):
            ps = psp.tile([128, 128], f32, tag="tp", name=f"tp_{tag}_{a}_{d}_{s}")
            nc.tensor.transpose(ps[:, :sz], src[:sz, d * 128:(d + 1) * 128], ident[:sz, :sz])
            if out32:
                nc.vector.tensor_copy(t32[d][:, a:b], ps[:, :sz])
            if out16:
                nc.scalar.copy(t16[d][:, a:b], ps[:, :sz])
    return t16, t32


def build(nc):
    x4_d = nc.dram_tensor("x4", [NSAMP, N, D], f32, kind="ExternalInput")
    psT_d = nc.dram_tensor("psT", [NSAMP, D, NPS], f16, kind="ExternalInput")
    gis_d = nc.dram_tensor("gis", [NSAMP, NS], f32, kind="ExternalInput")
    wqkv16_d = nc.dram_tensor("wqkv16", [D, 3 * D], f16, kind="ExternalInput")
    wqk32_d = nc.dram_tensor("wqk32", [D, 2 * D], f32, kind="ExternalInput")
    wproj16_d = nc.dram_tensor("wproj16", [D, D], f16, kind="ExternalInput")
    wq16_d = nc.dram_tensor("wq16", [D, D], f16, kind="ExternalInput")
    wk16_d = nc.dram_tensor("wk16", [D, D], f16, kind="ExternalInput")
    wv16_d = nc.dram_tensor("wv16", [D, D], f16, kind="ExternalInput")
    wtp16_d = nc.dram_tensor("wtp16", [D, D], f16, kind="ExternalInput")
    w116_d = nc.dram_tensor("w116", [D, 4 * D], f16, kind="ExternalInput")
    w216_d = nc.dram_tensor("w216", [4 * D, D], f16, kind="ExternalInput")
    ident_d = nc.dram_tensor("ident", [128, 128], f32, kind="ExternalInput")
    iota_d = nc.dram_tensor("iota", [128, NS], f32, kind="ExternalInput")
    lt0_d = nc.dram_tensor("lt0", [128, NS], f32, kind="ExternalInput")
    lt1_d = nc.dram_tensor("lt1", [128, NS], f32, kind="ExternalInput")

    attn_o = nc.dram_tensor("attn_o", [NSAMP, H, N, N], f32, kind="ExternalOutput")
    x_o = nc.dram_tensor("x_o", [NSAMP, NK, D], f32, kind="ExternalOutput")
    idx_o = nc.dram_tensor("idx_o", [NSAMP, NS], i32, kind="ExternalOutput")

    at_scr = nc.dram_tensor("at_scr", [NSAMP, NS], f32, kind="Internal")
    spill = nc.dram_tensor("spill", [NSAMP, NK, D], f32, kind="Internal")

    with TileContext(nc) as tc:
        with tc.tile_pool(name="consts", bufs=1) as cpool:
            ident = cpool.tile([128, 128], f32, tag="ident")
            nc.sync.dma_start(out=ident, in_=ident_d[:, :])
            iota_b = cpool.tile([128, NS], f32, tag="iota")
            nc.sync.dma_start(out=iota_b, in_=iota_d[:, :])
            lt_sb = [cpool.tile([128, NS], f32, tag=f"lt{t}", name=f"lt{t}") for t in range(2)]
            nc.sync.dma_start(out=lt_sb[0], in_=lt0_d[:, :])
            nc.sync.dma_start(out=lt_sb[1], in_=lt1_d[:, :])
            ones16c = cpool.tile([128, 1], f16, tag="ones16c")
            nc.vector.memset(ones16c, 1.0)
            ones16r = cpool.tile([1, 128], f16, tag="ones16r")
            nc.vector.memset(ones16r, 1.0)
            ones32c = cpool.tile([128, 1], f32, tag="ones32c")
            nc.vector.memset(ones32c, 1.0)
            eps_t = cpool.tile([128, 1], f32, tag="eps_t")
            nc.vector.memset(eps_t, EPS)

            # ---------------- STAGE A ----------------
            with tc.tile_pool(name="wA", bufs=1) as wA, \
                 tc.tile_pool(name="actA", bufs=1) as pa, \
                 tc.tile_pool(name="psA", bufs=2, space="PSUM") as psp:
                # psum tags: tp(x2), big(x3), sm(x3) = 8 banks
                _ptc = [0]
                def PT(shape, tag):
                    _ptc[0] += 1
                    return psp.tile(shape, f32, tag=tag, name=f"psA_{tag}_{_ptc[0]}")

                wqkv16 = [wA.tile([128, 3 * D], f16, tag=f"wqkv16_{d}", name=f"wqkv16_{d}") for d in range(ND)]
                wqk32 = [wA.tile([128, 2 * D], f32, tag=f"wqk32_{d}", name=f"wqk32_{d}") for d in range(ND)]
                wproj16 = [wA.tile([128, D], f16, tag=f"wproj16_{d}", name=f"wproj16_{d}") for d in range(ND)]
                for d in range(ND):
                    nc.sync.dma_start(out=wqkv16[d], in_=wqkv16_d[d * 128:(d + 1) * 128, :])
                    nc.sync.dma_start(out=wqk32[d], in_=wqk32_d[d * 128:(d + 1) * 128, :])
                    nc.sync.dma_start(out=wproj16[d], in_=wproj16_d[d * 128:(d + 1) * 128, :])

                for s in range(NSAMP):
                    xt = []
                    for i, (a, b) in enumerate(TT):
                        t = pa.tile([128, D], f32, tag=f"x{i}", name=f"x{i}_{s}")
                        nc.sync.dma_start(out=t[:b - a, :], in_=x4_d[s, a:b, :])
                        xt.append(t)
                    sizes = [b - a for a, b in TT]
                    ht = _ln_normalize(nc, pa, xt, sizes, "ln1", s, eps_t)
                    hT16, hT32 = _transpose_cols(nc, pa, psp, ident, ht, TT, N, "hT", s,
                                                 out16=True, out32=True)
                    kT32 = [pa.tile([128, N], f32, tag=f"kT32_{d}", name=f"kT32_{d}_{s}") for d in range(ND)]
                    kT16 = [pa.tile([128, N], f16, tag=f"kT16_{d}", name=f"kT16_{d}_{s}") for d in range(ND)]
                    for m in range(ND):
                        ps = PT([128, N], "big")
                        for kt in range(ND):
                            nc.tensor.matmul(ps, wqk32[kt][:, D + m * 128:D + (m + 1) * 128],
                                             hT32[kt], start=(kt == 0), stop=(kt == ND - 1))
                        nc.vector.tensor_copy(kT32[m], ps)
                        nc.scalar.copy(kT16[m], ps)
                    qT16 = [pa.tile([128, N], f16, tag=f"qT16_{d}", name=f"qT16_{d}_{s}") for d in range(ND)]
                    for m in range(ND):
                        ps = PT([128, N], "big")
                        for kt in range(ND):
                            nc.tensor.matmul(ps, wqkv16[kt][:, m * 128:(m + 1) * 128],
                                             hT16[kt], start=(kt == 0), stop=(kt == ND - 1))
                        nc.vector.tensor_copy(qT16[m], ps)
                    qtT32 = [pa.tile([128, NT], f32, tag=f"qtT32_{d}", name=f"qtT32_{d}_{s}") for d in range(ND)]
                    for m in range(ND):
                        ps = PT([128, NT], "sm")
                        for kt in range(ND):
                            nc.tensor.matmul(ps, wqk32[kt][:, m * 128:(m + 1) * 128],
                                             hT32[kt][:, 0:NT], start=(kt == 0), stop=(kt == ND - 1))
                        nc.vector.tensor_copy(qtT32[m], ps)
                    v16 = [pa.tile([128, D], f16, tag=f"v16_{t}", name=f"v16_{t}_{s}") for t in range(3)]
                    for t, (a, b) in enumerate(KT):
                        for c0, c1 in [(0, 384), (384, 768)]:
                            ps = PT([128, 384], "big")
                            for kt in range(ND):
                                nc.tensor.matmul(ps[:b - a, :], hT16[kt][:, a:b],
                                                 wqkv16[kt][:, 2 * D + c0:2 * D + c1],
                                                 start=(kt == 0), stop=(kt == ND - 1))
                            nc.vector.tensor_copy(v16[t][:b - a, c0:c1], ps[:b - a, :])

                    A_acc = pa.tile([NT, NS], f32, tag="A_acc", name=f"A_acc_{s}")
                    xaT16 = [pa.tile([128, N], f16, tag=f"xaT16_{d}", name=f"xaT16_{d}_{s}") for d in range(ND)]
                    for h in range(H):
                        dt_i, off = h // 2, (h % 2) * 64
                        kT32_h = kT32[dt_i][off:off + 64, :]
                        kT16_h = kT16[dt_i][off:off + 64, :]
                        qT16_h = qT16[dt_i][off:off + 64, :]
                        qtT32_h = qtT32[dt_i][off:off + 64, :]
                        for qt, (a, b) in enumerate(KT):
                            sz = b - a
                            ps = PT([128, N], "big")
                            if qt == 0:
                                nc.tensor.matmul(ps[0:64, :], qtT32_h, kT32_h, start=True, stop=True)
                                nc.tensor.matmul(ps[64:128, :], qT16_h[:, 64:128], kT16_h,
                                                 start=True, stop=True)
                            else:
                                nc.tensor.matmul(ps[:sz, :], qT16_h[:, a:b], kT16_h,
                                                 start=True, stop=True)
                            at = pa.tile([128, N], f32, tag=f"attn{qt}", name=f"attn{qt}_{h}_{s}")
                            den = pa.tile([128, 1], f32, tag=f"den{qt}", name=f"den{qt}_{h}_{s}")
                            nc.scalar.activation(at[:sz], ps[:sz], AF.Exp, scale=SCALE,
                                                 accum_out=den[:sz])
                            rec = pa.tile([128, 1], f32, tag=f"rec{qt}", name=f"rec{qt}_{h}_{s}")
                            nc.vector.reciprocal(rec[:sz], den[:sz])
                            nc.vector.tensor_scalar(at[:sz], at[:sz], rec[:sz], scalar2=None,
                                                    op0=ALU.mult)
                            nc.sync.dma_start(out=attn_o[s, h, a:b, :], in_=at[:sz])
                            if qt == 0:
                                if h == 0:
                                    nc.vector.tensor_copy(A_acc, at[0:NT, NT:N])
                                else:
                                    nc.vector.tensor_tensor(out=A_acc, in0=A_acc,
                                                            in1=at[0:NT, NT:N], op=ALU.add)
                        expT = []
                        for kt, (a, b) in enumerate(KT):
                            sz = b - a
                            ps = PT([128, N], "big")
                            nc.tensor.matmul(ps[:sz, :], kT16_h[:, a:b], qT16_h, start=True, stop=True)
                            e16 = pa.tile([128, N], f16, tag=f"expT{kt}", name=f"expT{kt}_{h}_{s}")
                            nc.scalar.activation(e16[:sz], ps[:sz], AF.Exp, scale=SCALE)
                            expT.append(e16)
                        psD = PT([1, N], "sm")
                        for kt, (a, b) in enumerate(KT):
                            sz = b - a
                            nc.tensor.matmul(psD, ones16c[0:sz, 0:1], expT[kt][:sz, :],
                                             start=(kt == 0), stop=(kt == 2))
                        recT16 = pa.tile([1, N], f16, tag="recT16", name=f"recT16_{h}_{s}")
                        recT = pa.tile([1, N], f32, tag="recT", name=f"recT_{h}_{s}")
                        nc.vector.reciprocal(recT, psD)
                        nc.vector.tensor_copy(recT16, recT)
                        psX = PT([64, N], "sm")
                        for kt, (a, b) in enumerate(KT):
                            sz = b - a
                            nc.tensor.matmul(psX, v16[kt][:sz, h * 64:(h + 1) * 64],
                                             expT[kt][:sz, :], start=(kt == 0), stop=(kt == 2))
                        psB = PT([64, N], "sm")
                        nc.tensor.matmul(psB, ones16r[0:1, 0:64], recT16, start=True, stop=True)
                        recb = pa.tile([64, N], f32, tag="recb", name=f"recb_{h}_{s}")
                        nc.scalar.copy(recb, psB)
                        nc.vector.tensor_tensor(out=xaT16[dt_i][off:off + 64, :], in0=psX,
                                                in1=recb, op=ALU.mult)

                    for t, (a, b) in enumerate(TT):
                        sz = b - a
                        for c0, c1 in [(0, 384), (384, 768)]:
                            ps = PT([128, 384], "big")
                            for kt in range(ND):
                                nc.tensor.matmul(ps[:sz, :], xaT16[kt][:, a:b],
                                                 wproj16[kt][:, c0:c1],
                                                 start=(kt == 0), stop=(kt == ND - 1))
                            nc.vector.tensor_tensor(out=xt[t][:sz, c0:c1], in0=xt[t][:sz, c0:c1],
                                                    in1=ps[:sz, :], op=ALU.add)

                    psS = PT([1, NS], "sm")
                    nc.tensor.matmul(psS, ones32c[0:NT, 0:1], A_acc, start=True, stop=True)
                    arow = pa.tile([1, NS], f32, tag="arow", name=f"arow_{s}")
                    nc.vector.tensor_scalar(arow, psS, 1.0 / (H * NT), scalar2=None, op0=ALU.mult)
                    nc.sync.dma_start(out=at_scr[s:s + 1, :], in_=arow)
                    vb = pa.tile([128, NS], f32, tag="vb", name=f"vb_{s}")
                    nc.sync.dma_start(out=vb, in_=at_scr[s:s + 1, :].partition_broadcast(128))
                    oh32 = [pa.tile([128, NS], f32, tag=f"oh{t}", name=f"oh{t}_{s}") for t in range(2)]
                    gis_c = [pa.tile([128, 1], f32, tag=f"gisc{t}", name=f"gisc{t}_{s}") for t in range(2)]
                    for t in range(2):
                        vcol = pa.tile([128, 1], f32, tag="vcol", name=f"vcol{t}_{s}")
                        nc.sync.dma_start(out=vcol, in_=at_scr[s:s + 1, t * 128:(t + 1) * 128]
                                          .rearrange("one (p f) -> (one p) f", p=128))
                        nc.sync.dma_start(out=gis_c[t], in_=gis_d[s:s + 1, t * 128:(t + 1) * 128]
                                          .rearrange("one (p f) -> (one p) f", p=128))
                        gtt = pa.tile([128, NS], f32, tag="gtt", name=f"gtt{t}_{s}")
                        nc.vector.tensor_scalar(gtt, vb, vcol, scalar2=None, op0=ALU.is_gt)
                        r_gt = pa.tile([128, 1], f32, tag="rgt", name=f"rgt{t}_{s}")
                        nc.vector.tensor_reduce(r_gt, gtt, axis=AX.X, op=ALU.add)
                        eq = pa.tile([128, NS], f32, tag="eq", name=f"eq{t}_{s}")
                        nc.vector.tensor_scalar(eq, vb, vcol, scalar2=None, op0=ALU.is_equal)
                        nc.vector.tensor_tensor(out=eq, in0=eq, in1=lt_sb[t], op=ALU.mult)
                        r_eq = pa.tile([128, 1], f32, tag="req", name=f"req{t}_{s}")
                        nc.vector.tensor_reduce(r_eq, eq, axis=AX.X, op=ALU.add)
                        rank = pa.tile([128, 1], f32, tag="rank", name=f"rank{t}_{s}")
                        nc.vector.tensor_tensor(out=rank, in0=r_gt, in1=r_eq, op=ALU.add)
                        nc.vector.tensor_scalar(oh32[t], iota_b, rank, scalar2=None, op0=ALU.is_equal)
                    for mt in range(2):
                        psI = PT([128, 1], "sm")
                        for kt in range(2):
                            nc.tensor.matmul(psI, oh32[kt][:, mt * 128:(mt + 1) * 128], gis_c[kt],
                                             start=(kt == 0), stop=(kt == 1))
                        ri = pa.tile([128, 1], i32, tag="ri", name=f"ri{mt}_{s}")
                        nc.vector.tensor_copy(ri, psI)
                        nc.sync.dma_start(out=idx_o[s:s + 1, mt * 128:(mt + 1) * 128]
                                          .rearrange("one (p f) -> (one p) f", p=128), in_=ri)
                        for c0, c1 in [(0, 512), (512, 768)]:
                            psG = PT([128, 512], "big")
                            for kt in range(2):
                                nc.tensor.matmul(psG[:, 0:c1 - c0],
                                                 oh32[kt][:, mt * 128:(mt + 1) * 128],
                                                 xt[kt + 1][:, c0:c1],
                                                 start=(kt == 0), stop=(kt == 1))
                            go = pa.tile([128, 512], f32, tag="go", name=f"go{mt}_{c0}_{s}")
                            nc.vector.tensor_copy(go[:, 0:c1 - c0], psG[:, 0:c1 - c0])
                            if mt == 0:
                                nc.sync.dma_start(out=spill[s, 64:192, c0:c1], in_=go[:, 0:c1 - c0])
                            else:
                                nc.sync.dma_start(out=spill[s, 192:244, c0:c1], in_=go[0:52, 0:c1 - c0])
                    nc.sync.dma_start(out=spill[s, 0:64, :], in_=xt[0][0:64, :])

            # ---------------- STAGE B ----------------
            with tc.tile_pool(name="wB", bufs=1) as wB, \
                 tc.tile_pool(name="actB", bufs=1) as pb, \
                 tc.tile_pool(name="psB", bufs=2, space="PSUM") as pspb:
                _pbc = [0]
                def PB(shape, tag):
                    _pbc[0] += 1
                    return pspb.tile(shape, f32, tag=tag, name=f"psB_{tag}_{_pbc[0]}")

                wq16 = [wB.tile([128, D], f16, tag=f"wq16_{d}", name=f"wq16_{d}") for d in range(ND)]
                wk16 = [wB.tile([128, D], f16, tag=f"wk16_{d}", name=f"wk16_{d}") for d in range(ND)]
                wv16 = [wB.tile([128, D], f16, tag=f"wv16_{d}", name=f"wv16_{d}") for d in range(ND)]
                wtp16 = [wB.tile([128, D], f16, tag=f"wtp16_{d}", name=f"wtp16_{d}") for d in range(ND)]
                for d in range(ND):
                    nc.sync.dma_start(out=wq16[d], in_=wq16_d[d * 128:(d + 1) * 128, :])
                    nc.sync.dma_start(out=wk16[d], in_=wk16_d[d * 128:(d + 1) * 128, :])
                    nc.sync.dma_start(out=wv16[d], in_=wv16_d[d * 128:(d + 1) * 128, :])
                    nc.sync.dma_start(out=wtp16[d], in_=wtp16_d[d * 128:(d + 1) * 128, :])

                for s in range(NSAMP):
                    st_t = []
                    for i, (a, b) in enumerate(ST):
                        t = pb.tile([128, D], f32, tag=f"s{i}", name=f"s{i}_{s}")
                        nc.sync.dma_start(out=t[:b - a, :], in_=spill[s, a:b, :])
                        st_t.append(t)
                    psT16 = [pb.tile([128, NPS], f16, tag=f"psT16_{d}", name=f"psT16_{d}_{s}") for d in range(ND)]
                    for d in range(ND):
                        nc.sync.dma_start(out=psT16[d], in_=psT_d[s, d * 128:(d + 1) * 128, :])
                    snt = _ln_normalize(nc, pb, st_t, [b - a for a, b in ST], "lnt", s, eps_t)
                    snT16, _ = _transpose_cols(nc, pb, pspb, ident, snt, SL, LENS_KEEP,
                                               "snT", s, out16=True, out32=False)
                    qT2 = [pb.tile([128, LENS_KEEP], f16, tag=f"qT2_{d}", name=f"qT2_{d}_{s}") for d in range(ND)]
                    kT2 = [pb.tile([128, NPS], f16, tag=f"kT2_{d}", name=f"kT2_{d}_{s}") for d in range(ND)]
                    for m in range(ND):
                        ps = PB([128, LENS_KEEP], "big")
                        for kt in range(ND):
                            nc.tensor.matmul(ps, wq16[kt][:, m * 128:(m + 1) * 128], snT16[kt],
                                             start=(kt == 0), stop=(kt == ND - 1))
                        nc.vector.tensor_copy(qT2[m], ps)
                        ps2 = PB([128, NPS], "big")
                        for kt in range(ND):
                            nc.tensor.matmul(ps2, wk16[kt][:, m * 128:(m + 1) * 128], psT16[kt],
                                             start=(kt == 0), stop=(kt == ND - 1))
                        nc.vector.tensor_copy(kT2[m], ps2)
                    vT2 = [pb.tile([128, D], f16, tag=f"vT2_{t}", name=f"vT2_{t}_{s}") for t in range(2)]
                    for t in range(2):
                        for c0, c1 in [(0, 384), (384, 768)]:
                            ps = PB([128, 384], "big")
                            for kt in range(ND):
                                nc.tensor.matmul(ps, psT16[kt][:, t * 128:(t + 1) * 128],
                                                 wv16[kt][:, c0:c1],
                                                 start=(kt == 0), stop=(kt == ND - 1))
                            nc.vector.tensor_copy(vT2[t][:, c0:c1], ps)
                    saT16 = [pb.tile([128, LENS_KEEP], f16, tag=f"saT16_{d}", name=f"saT16_{d}_{s}")
                             for d in range(ND)]
                    for h in range(H):
                        dt_i, off = h // 2, (h % 2) * 64
                        kT2_h = kT2[dt_i][off:off + 64, :]
                        qT2_h = qT2[dt_i][off:off + 64, :]
                        expT = []
                        for kt in range(2):
                            ps = PB([128, LENS_KEEP], "big")
                            nc.tensor.matmul(ps, kT2_h[:, kt * 128:(kt + 1) * 128], qT2_h,
                                             start=True, stop=True)
                            e16 = pb.tile([128, LENS_KEEP], f16, tag=f"expT2{kt}",
                                          name=f"expT2{kt}_{h}_{s}")
                            nc.scalar.activation(e16, ps, AF.Exp, scale=SCALE)
                            expT.append(e16)
                        psD = PB([1, LENS_KEEP], "sm")
                        for kt in range(2):
                            nc.tensor.matmul(psD, ones16c[0:128, 0:1], expT[kt],
                                             start=(kt == 0), stop=(kt == 1))
                        recT = pb.tile([1, LENS_KEEP], f32, tag="recT2", name=f"recT2_{h}_{s}")
                        nc.vector.reciprocal(recT, psD)
                        recT16 = pb.tile([1, LENS_KEEP], f16, tag="recT216", name=f"recT216_{h}_{s}")
                        nc.vector.tensor_copy(recT16, recT)
                        psX = PB([64, LENS_KEEP], "sm")
                        for kt in range(2):
                            nc.tensor.matmul(psX, vT2[kt][:, h * 64:(h + 1) * 64], expT[kt],
                                             start=(kt == 0), stop=(kt == 1))
                        psBb = PB([64, LENS_KEEP], "sm")
                        nc.tensor.matmul(psBb, ones16r[0:1, 0:64], recT16, start=True, stop=True)
                        recb = pb.tile([64, LENS_KEEP], f32, tag="recb2", name=f"recb2_{h}_{s}")
                        nc.scalar.copy(recb, psBb)
                        nc.vector.tensor_tensor(out=saT16[dt_i][off:off + 64, :], in0=psX,
                                                in1=recb, op=ALU.mult)
                    for t, (a, b) in enumerate(SL):
                        sz = b - a
                        for c0, c1 in [(0, 384), (384, 768)]:
                            ps = PB([128, 384], "big")
                            for kt in range(ND):
                                nc.tensor.matmul(ps[:sz, :], saT16[kt][:, a:b],
                                                 wtp16[kt][:, c0:c1],
                                                 start=(kt == 0), stop=(kt == ND - 1))
                            nc.vector.tensor_tensor(out=st_t[t][:sz, c0:c1],
                                                    in0=st_t[t][:sz, c0:c1],
                                                    in1=ps[:sz, :], op=ALU.add)
                    for i, (a, b) in enumerate(ST):
                        nc.sync.dma_start(out=spill[s, a:b, :], in_=st_t[i][:b - a, :])

            # ---------------- STAGE C ----------------
            with tc.tile_pool(name="wC", bufs=1) as wC, \
                 tc.tile_pool(name="actC", bufs=1) as pc, \
                 tc.tile_pool(name="psC", bufs=2, space="PSUM") as pspc:
                w116 = [wC.tile([128, 4 * D], f16, tag=f"w116_{d}", name=f"w116_{d}") for d in range(ND)]
                for d in range(ND):
                    nc.sync.dma_start(out=w116[d], in_=w116_d[d * 128:(d + 1) * 128, :])
                w216 = [wC.tile([128, D], f16, tag=f"w216_{d}", name=f"w216_{d}") for d in range(4 * ND)]
                for d in range(4 * ND):
                    nc.sync.dma_start(out=w216[d], in_=w216_d[d * 128:(d + 1) * 128, :])

                for s in range(NSAMP):
                    yt = []
                    for i, (a, b) in enumerate(YT):
                        t = pc.tile([128, D], f32, tag=f"y{i}", name=f"y{i}_{s}")
                        nc.sync.dma_start(out=t[:b - a, :], in_=spill[s, a:b, :])
                        yt.append(t)
                    h2 = _ln_normalize(nc, pc, yt, [b - a for a, b in YT], "ln2", s, eps_t)
                    h2T16, _ = _transpose_cols(nc, pc, pspc, ident, h2, YT, NK,
                                               "h2T", s, out16=True, out32=False)
                    g16 = [pc.tile([128, NK], f16, tag=f"g16_{m}", name=f"g16_{m}_{s}")
                           for m in range(4 * ND)]
                    for m in range(4 * ND):
                        ps = pspc.tile([128, NK], f32, tag="big", name=f"psC1_{m}_{s}")
                        for kt in range(ND):
                            nc.tensor.matmul(ps, w116[kt][:, m * 128:(m + 1) * 128], h2T16[kt],
                                             start=(kt == 0), stop=(kt == ND - 1))
                        nc.scalar.activation(g16[m], ps, AF.Gelu)
                    for t, (a, b) in enumerate(YT):
                        sz = b - a
                        for c0, c1 in [(0, 384), (384, 768)]:
                            ps = pspc.tile([128, 384], f32, tag="big2", name=f"psC2_{t}_{c0}_{s}")
                            for kt in range(4 * ND):
                                nc.tensor.matmul(ps[:sz, :], g16[kt][:, a:b], w216[kt][:, c0:c1],
                                                 start=(kt == 0), stop=(kt == 4 * ND - 1))
                            nc.vector.tensor_tensor(out=yt[t][:sz, c0:c1], in0=yt[t][:sz, c0:c1],
                                                    in1=ps[:sz, :], op=ALU.add)
                    for i, (a, b) in enumerate(YT):
                        nc.sync.dma_start(out=x_o[s, a:b, :], in_=yt[i][:b - a, :])
    return nc


_BUILT = None
_LAST = None


def _get_built():
    global _BUILT
    if _BUILT is None:
        nc = bacc.Bacc("TRN2", target_bir_lowering=False)
        build(nc)
        nc.finalize()
        _BUILT = nc
    return _BUILT


def kernel(**inputs):
    inp = {k: np.asarray(v) for k, v in inputs.items()}
    x = inp["x"].astype(np.float32)
    ps = inp["ps"].astype(np.float32)
    git = inp["global_index_template"]
    gis = inp["global_index_search"]
    g1, gt, g2 = inp["g1"], inp["gt"], inp["g2"]
    for z in ("b1", "bproj", "bt", "btp", "b1m", "b2", "b2m"):
        assert np.abs(inp[z]).max() == 0.0, f"{z} nonzero; kernel assumes zero biases"
    wqkv = inp["Wqkv"].astype(np.float32) * g1[:, None]
    wq = inp["Wq"].astype(np.float32) * gt[:, None]
    w1 = inp["W1"].astype(np.float32) * g2[:, None]

    ident = np.eye(128, dtype=np.float32)
    iota = np.broadcast_to(np.arange(NS, dtype=np.float32), (128, NS)).copy()
    lt0 = (np.arange(NS)[None, :] < np.arange(0, 128)[:, None]).astype(np.float32)
    lt1 = (np.arange(NS)[None, :] < np.arange(128, 256)[:, None]).astype(np.float32)

    shared = {
        "wqkv16": wqkv.astype(np.float16),
        "wqk32": np.ascontiguousarray(wqkv[:, 0:2 * D]),
        "wproj16": inp["Wproj"].astype(np.float16),
        "wq16": wq.astype(np.float16),
        "wk16": inp["Wk"].astype(np.float16),
        "wv16": inp["Wv"].astype(np.float16),
        "wtp16": inp["Wtp"].astype(np.float16),
        "w116": w1.astype(np.float16),
        "w216": inp["W2"].astype(np.float16),
        "ident": ident, "iota": iota, "lt0": lt0, "lt1": lt1,
    }
    in_maps = []
    for c in range(8):
        sl = slice(c * NSAMP, (c + 1) * NSAMP)
        m = dict(shared)
        m["x4"] = np.ascontiguousarray(x[sl])
        m["psT"] = np.ascontiguousarray(ps[sl].transpose(0, 2, 1)).astype(np.float16)
        m["gis"] = gis[sl].astype(np.float32)
        in_maps.append(m)

    nc = _get_built()
    _res_obj = bass_utils.run_bass_kernel_spmd(nc, in_maps, core_ids=list(range(8)))
    global _LAST
    _LAST = _res_obj
    results = _res_obj.results

    x_out = np.concatenate([r["x_o"] for r in results], axis=0)
    attn = np.concatenate([r["attn_o"] for r in results], axis=0)
    idx = np.concatenate([r["idx_o"] for r in results], axis=0)
    keep = idx[:, :LENS_KEEP].astype(gis.dtype)
    removed = idx[:, LENS_KEEP:].astype(gis.dtype)
    return (x_out, git, keep, removed, attn)
